# revision 19
# baseline (speedup 1.0000x reference)
"""Multi-head attention (B=4, S=2048, D=1024, H=16) on 8 TRN2 NeuronCores.

Sharding: 2D grid batch(4) x head-group(2). Core c handles batch c//2 and
heads [ (c%2)*8 , (c%2)*8+8 ). Each core computes:
  - Q,K projections for its 8 heads in transposed layout q.T/k.T [512, 2048]
    (spilled to DRAM scratch, reloaded per 2-head group)
  - V projection in natural layout [2048, 512], stored ones-augmented per head
    ([128, 16, 8, 65] with col 64 = 1.0) so the AV matmul also produces the
    softmax denominators
  - attention per head: S.T = K Q^T (scores transposed), exp via ACT (scale
    fused), AV matmul accumulating C.T[hd,sq] + denom row, normalize via
    reciprocal + gpsimd partition-broadcast
  - output projection out.T = Wo_p.T^T @ ctx.T accumulated over head groups
Host side: shard/rearrange inputs, sum the two partial outputs per batch,
add bo, transpose back.

Matmul dtype: bfloat16 by default (KERNEL_DTYPE=f32r for float32r fallback).
PSUM accumulation is always fp32; biases and normalization are fp32.
"""
import os
import sys

sys.path.insert(0, "/opt/trn_rl_repo")

import types

# antenv.axon_hooks is missing from this image; install a shim so
# run_bass_kernel_spmd(trace=True) can reach the NTFF profiler.
if "antenv.axon_hooks" not in sys.modules:
    _mod = types.ModuleType("antenv.axon_hooks")
    _hook = [None]
    _mod.set_axon_ntff_profile_hook = lambda h: _hook.__setitem__(0, h)
    _mod.get_axon_ntff_profile_hook = lambda: _hook[0]
    sys.modules["antenv.axon_hooks"] = _mod
    import antenv

    antenv.axon_hooks = _mod
    try:
        from trn_agent_boot.trn_boot import _ntff_profile_via_ctypes

        _mod.set_axon_ntff_profile_hook(
            _ntff_profile_via_ctypes("/opt/axon/libaxon_pjrt.so")
        )
    except Exception:
        pass

import ml_dtypes
import numpy as np
import concourse.bass as bass
import concourse.mybir as mybir
import concourse.tile as tile
from concourse import bacc
from concourse import bass_utils

B, S, D, H = 4, 2048, 1024, 16
HD = D // H  # 64
SCALE = HD ** -0.5
NCORES = 8
HPC = 8          # heads per core
JP = HPC * HD    # 512 projected feature cols per core
G = 4            # 2-head groups per core
DC = D // 128    # 8 contraction chunks
NST = S // 128   # 16 s-tiles
F32 = mybir.dt.float32
F32R = mybir.dt.float32r

DTYPE_NAME = os.environ.get("KERNEL_DTYPE", "bf16")
if DTYPE_NAME == "bf16":
    MDT = mybir.dt.bfloat16
    NPDT = ml_dtypes.bfloat16
    SBP = 512    # s-block for projections
    PAD = True   # zero-pad QK contraction + AV stationary to full 128x128
else:
    MDT = F32R
    NPDT = np.float32
    SBP = 256
    PAD = False
NSB = S // SBP
VW = 128 if PAD else HD + 1  # v' head stride
AVP = 128 if PAD else HD + 1  # av psum partitions

_CACHE = {}


def _build():
    nc = bacc.Bacc("TRN2", target_bir_lowering=False, debug=False)

    xq = nc.dram_tensor("xq", [NSB, 128, DC, SBP], MDT, kind="ExternalInput").ap()
    xk = nc.dram_tensor("xk", [NSB, 128, DC, SBP], MDT, kind="ExternalInput").ap()
    xv = nc.dram_tensor("xv", [NSB, 128, DC, SBP], MDT, kind="ExternalInput").ap()
    wq = nc.dram_tensor("wq", [128, DC, JP], MDT, kind="ExternalInput").ap()
    wk = nc.dram_tensor("wk", [128, DC, JP], MDT, kind="ExternalInput").ap()
    wv = nc.dram_tensor("wv", [128, DC, JP], MDT, kind="ExternalInput").ap()
    wo = nc.dram_tensor("wo", [128, G, D], MDT, kind="ExternalInput").ap()
    bq = nc.dram_tensor("bq", [128, 4], F32, kind="ExternalInput").ap()
    bk = nc.dram_tensor("bk", [128, 4], F32, kind="ExternalInput").ap()
    bv = nc.dram_tensor("bv", [1, JP], F32, kind="ExternalInput").ap()
    out = nc.dram_tensor("out", [D, S], F32, kind="ExternalOutput").ap()

    with tile.TileContext(nc) as tc:
        with (
            tc.tile_pool(name="consts", bufs=1) as consts,
            tc.tile_pool(name="wts", bufs=3) as wts,
            tc.tile_pool(name="xs", bufs=2) as xs,
            tc.tile_pool(name="stage", bufs=3) as stage,
            tc.tile_pool(name="vpool", bufs=1) as vpool,
            tc.tile_pool(name="qk", bufs=2) as qkp,
            tc.tile_pool(name="ex", bufs=3) as exp_pool,
            tc.tile_pool(name="nrm", bufs=3) as nrm,
            tc.tile_pool(name="ctxp", bufs=4) as ctxp,
            tc.tile_pool(name="outst", bufs=3) as outst,
            tc.tile_pool(name="scr", bufs=1, space="DRAM") as scr,
        ):
            if not PAD:
                qsc = scr.tile([G, 128, S], MDT, tag="qsc")
                ksc = scr.tile([G, 128, S], MDT, tag="ksc")

            bq_sb = consts.tile([128, 4], F32, tag="bq")
            nc.gpsimd.dma_start(out=bq_sb[:], in_=bq[:])
            bk_sb = consts.tile([128, 4], F32, tag="bk")
            nc.gpsimd.dma_start(out=bk_sb[:], in_=bk[:])
            bv_row = consts.tile([1, JP], F32, tag="bvr")
            nc.gpsimd.dma_start(out=bv_row[:], in_=bv[:])
            bv_bc = consts.tile([128, JP], F32, tag="bvb")
            nc.gpsimd.partition_broadcast(bv_bc[:], bv_row[:])

            # V' resident tile: [p, s_tile, head, VW]; col 64 = ones, cols
            # 65.. = zero pad (PAD mode keeps the PE array fully occupied and
            # FWL-eligible).
            v_sb = vpool.tile([128, NST, HPC, VW], MDT, tag="v")
            if PAD:
                nc.vector.memset(v_sb[:], 0.0)
            if MDT == F32R:
                nc.vector.memset(v_sb[:, :, :, HD:HD + 1].bitcast(F32), 1.0)
            else:
                nc.vector.memset(v_sb[:, :, :, HD:HD + 1], 1.0)

            # PAD mode: per-head zero-padded K tiles + resident q, written
            # directly by the projection phase (no DRAM scratch roundtrip).
            if PAD:
                k_all = vpool.tile([128, HPC, S], MDT, tag="kall")
                nc.vector.memset(k_all[:], 0.0)
                q_all = vpool.tile([128, G, S], MDT, tag="qall")

            # ------- Q/K/V projections, interleaved per s-block -------
            # Projections and attention share one PSUM pool (tags st/av) so
            # the scheduler can overlap the proj tail with early attention.
            mm = tc.tile_pool(name="mm", bufs=1, space="PSUM")
            pps = mm.__enter__()
            if True:
                w_sbs = {}
                w_drams = {"q": wq, "k": wk, "v": wv}
                for pname in ("q", "k", "v"):
                    w_sbs[pname] = wts.tile([128, DC, JP], MDT, tag="w",
                                            name=f"w{pname}_sb")
                for sb in range(NSB):
                    for pname, xdram, bias_sb, scratch in (
                        ("q", xq, bq_sb, None if PAD else qsc),
                        ("k", xk, bk_sb, None if PAD else ksc),
                        ("v", xv, None, None),
                    ):
                        x_sb = xs.tile([128, DC, SBP], MDT, tag="x",
                                       name=f"x{pname}_{sb}", bufs=6)
                        for dh in range(2):
                            nc.sync.dma_start(
                                out=x_sb[:, dh * 4:(dh + 1) * 4, :],
                                in_=xdram[sb, :, dh * 4:(dh + 1) * 4, :])
                        if sb == 0:
                            # weights emitted after the first x block so the
                            # x stream wins the early DMA bandwidth
                            for dh in range(2):
                                nc.sync.dma_start(
                                    out=w_sbs[pname][:, dh * 4:(dh + 1) * 4, :],
                                    in_=w_drams[pname][:, dh * 4:(dh + 1) * 4, :])
                        if pname != "v":
                            for jt in range(4):
                                ps_t = pps.tile([128, SBP], F32,
                                                tag=("st" if pname == "q" else "av"),
                                                bufs=2,
                                                name=f"ps{pname}_{sb}_{jt}")
                                for dc in range(DC):
                                    nc.tensor.matmul(
                                        ps_t[:],
                                        w_sbs[pname][:, dc, jt * 128:(jt + 1) * 128],
                                        x_sb[:, dc, :],
                                        start=(dc == 0), stop=(dc == DC - 1),
                                    )
                                ssl = slice(sb * SBP, (sb + 1) * SBP)
                                if PAD and pname == "q":
                                    nc.vector.tensor_scalar_add(
                                        out=q_all[:, jt, ssl], in0=ps_t[:],
                                        scalar1=bias_sb[:, jt:jt + 1],
                                    )
                                elif PAD:
                                    nc.vector.tensor_scalar_add(
                                        out=k_all[0:64, 2 * jt, ssl],
                                        in0=ps_t[0:64, :],
                                        scalar1=bias_sb[0:64, jt:jt + 1],
                                    )
                                    nc.vector.tensor_scalar_add(
                                        out=k_all[64:128, 2 * jt + 1, ssl],
                                        in0=ps_t[64:128, :],
                                        scalar1=bias_sb[64:128, jt:jt + 1],
                                    )
                                else:
                                    st_t = stage.tile([128, SBP], MDT, tag="stg",
                                                      name=f"st{pname}_{sb}_{jt}")
                                    nc.vector.tensor_scalar_add(
                                        out=st_t[:], in0=ps_t[:],
                                        scalar1=bias_sb[:, jt:jt + 1],
                                    )
                                    nc.sync.dma_start(
                                        out=scratch[jt, :, ssl],
                                        in_=st_t[:],
                                    )
                        else:
                            for half in range(SBP // 128):
                                sti = sb * (SBP // 128) + half
                                ps_v = pps.tile([128, JP], F32,
                                                tag=("st" if sti % 2 else "av"),
                                                bufs=2,
                                                name=f"psv_{sti}")
                                for dc in range(DC):
                                    nc.tensor.matmul(
                                        ps_v[:],
                                        x_sb[:, dc, half * 128:(half + 1) * 128],
                                        w_sbs["v"][:, dc, :],
                                        start=(dc == 0), stop=(dc == DC - 1),
                                    )
                                nc.vector.tensor_add(
                                    out=v_sb[:, sti, :, 0:HD],
                                    in0=ps_v.rearrange("p (h d) -> p h d", h=HPC),
                                    in1=bv_bc.rearrange("p (h d) -> p h d", h=HPC),
                                )

            # ---------------- attention ----------------
            ctx_tiles = []
            if True:
                aps = pps
                for g in range(G):
                    if not PAD:
                        q_sb = qkp.tile([128, S], MDT, tag="qg", name=f"qg_{g}")
                        nc.sync.dma_start(out=q_sb[:], in_=qsc[g])
                        k_sb = qkp.tile([128, S], MDT, tag="kg", name=f"kg_{g}")
                        nc.sync.dma_start(out=k_sb[:], in_=ksc[g])
                    ctx_t = ctxp.tile([128, S], MDT, tag="ctx", name=f"ctx_{g}")
                    ctx_tiles.append(ctx_t)
                    for hh in range(2):
                        p0 = hh * HD
                        hloc = 2 * g + hh
                        for sqp in range(2):
                            q0 = sqp * 1024
                            av = aps.tile([AVP, 2, 512], F32, tag="av", bufs=2,
                                          name=f"av_{g}_{hh}_{sqp}")
                            for sk in range(NST):
                                st = aps.tile([128, 2, 512], F32, tag="st", bufs=2,
                                              name=f"stt_{g}_{hh}_{sqp}_{sk}")
                                for i in range(2):
                                    if PAD:
                                        lhs_k = k_all[:, hloc, sk * 128:(sk + 1) * 128]
                                        rhs_q = q_all[:, g, q0 + i * 512:q0 + (i + 1) * 512]
                                    else:
                                        lhs_k = k_sb[p0:p0 + HD, sk * 128:(sk + 1) * 128]
                                        rhs_q = q_sb[p0:p0 + HD,
                                                     q0 + i * 512:q0 + (i + 1) * 512]
                                    nc.tensor.matmul(
                                        st[:, i, :], lhs_k, rhs_q,
                                        start=True, stop=True,
                                    )
                                ex_t = exp_pool.tile([128, 2, 512], MDT, tag="ex",
                                                     name=f"ex_{g}_{hh}_{sqp}_{sk}")
                                nc.scalar.activation(
                                    out=ex_t[:], in_=st[:],
                                    func=mybir.ActivationFunctionType.Exp,
                                    scale=SCALE,
                                )
                                for i in range(2):
                                    nc.tensor.matmul(
                                        av[:, i, :],
                                        v_sb[:, sk, hloc, :],
                                        ex_t[:, i, :],
                                        start=(sk == 0), stop=(sk == NST - 1),
                                    )
                            for i in range(2):
                                dcp = nrm.tile([1, 512], F32, tag="dcp",
                                               name=f"dcp_{g}_{hh}_{sqp}_{i}")
                                nc.vector.tensor_copy(
                                    out=dcp[:], in_=av[HD:HD + 1, i, :])
                                recip = nrm.tile([1, 512], F32, tag="rc",
                                                 name=f"rc_{g}_{hh}_{sqp}_{i}")
                                nc.vector.reciprocal_approx_fast(
                                    recip[:], dcp[:])
                                bcast = nrm.tile([HD, 512], F32, tag="bc",
                                                 name=f"bc_{g}_{hh}_{sqp}_{i}")
                                nc.gpsimd.partition_broadcast(bcast[:], recip[:])
                                nc.vector.tensor_mul(
                                    out=ctx_t[p0:p0 + HD,
                                              q0 + i * 512:q0 + (i + 1) * 512],
                                    in0=av[0:HD, i, :],
                                    in1=bcast[:],
                                )

            # ---------------- output projection (same psum pool) -------------
            if True:
                wo_sb = wts.tile([128, G, D], MDT, tag="w", name="wo_sb")
                for dh in range(2):
                    nc.sync.dma_start(out=wo_sb[:, dh * 2:(dh + 1) * 2, :],
                                      in_=wo[:, dh * 2:(dh + 1) * 2, :])
                for et in range(D // 128):
                    for sb4 in range(4):
                        ps_o = pps.tile([128, 512], F32,
                                        tag=("st" if (et * 4 + sb4) % 2 else "av"),
                                        bufs=2,
                                        name=f"pso_{et}_{sb4}")
                        for g in range(G):
                            nc.tensor.matmul(
                                ps_o[:],
                                wo_sb[:, g, et * 128:(et + 1) * 128],
                                ctx_tiles[g][:, sb4 * 512:(sb4 + 1) * 512],
                                start=(g == 0), stop=(g == G - 1),
                            )
                        o_sb = outst.tile([128, 512], F32, tag="os", bufs=4,
                                          name=f"os_{et}_{sb4}")
                        nc.vector.tensor_copy(out=o_sb[:], in_=ps_o[:])
                        nc.sync.dma_start(
                            out=out[et * 128:(et + 1) * 128,
                                    sb4 * 512:(sb4 + 1) * 512],
                            in_=o_sb[:],
                        )
            mm.__exit__(None, None, None)
    nc.compile()
    return nc


def get_nc():
    if "nc" not in _CACHE:
        _CACHE["nc"] = _build()
    return _CACHE["nc"]


def _arrange_x(x):
    """[S, D] -> [NSB, 128, DC, SBP] with [sb, p, dc, s] = x[sb*SBP+s, dc*128+p]."""
    return np.ascontiguousarray(
        x.reshape(NSB, SBP, DC, 128).transpose(0, 3, 2, 1)
    ).astype(NPDT)


def _arrange_w(w_p):
    """W_p [JP, D] (rows for this core's heads) -> [128, DC, JP] lhsT layout."""
    # want [p, dc, j] = W_p.T[dc*128+p, j] = W_p[j, dc*128+p]
    return np.ascontiguousarray(
        w_p.T.reshape(DC, 128, JP).transpose(1, 0, 2)).astype(NPDT)


def _arrange_wo(wo_p):
    """Wo_p = Wo[:, cols] [D, JP] -> [128, G, D] with [p,g,e] = Wo_p[e, g*128+p]."""
    return np.ascontiguousarray(
        wo_p.T.reshape(G, 128, D).transpose(1, 0, 2)).astype(NPDT)


def prepare_in_maps(query, key, value, Wq, bq, Wk, bk, Wv, bv, Wo, bo):
    xs_arr = {}
    for b in range(B):
        xs_arr[("q", b)] = _arrange_x(np.asarray(query[b], np.float32))
        xs_arr[("k", b)] = _arrange_x(np.asarray(key[b], np.float32))
        xs_arr[("v", b)] = _arrange_x(np.asarray(value[b], np.float32))
    ws = {}
    for gidx in range(2):
        rows = slice(gidx * JP, (gidx + 1) * JP)
        ws[("wq", gidx)] = _arrange_w(np.asarray(Wq, np.float32)[rows])
        ws[("wk", gidx)] = _arrange_w(np.asarray(Wk, np.float32)[rows])
        ws[("wv", gidx)] = _arrange_w(np.asarray(Wv, np.float32)[rows])
        ws[("wo", gidx)] = _arrange_wo(np.asarray(Wo, np.float32)[:, rows])
        ws[("bq", gidx)] = np.ascontiguousarray(
            np.asarray(bq, np.float32)[rows].reshape(4, 128).T)
        ws[("bk", gidx)] = np.ascontiguousarray(
            np.asarray(bk, np.float32)[rows].reshape(4, 128).T)
        ws[("bv", gidx)] = np.asarray(bv, np.float32)[rows].reshape(1, JP).copy()
    in_maps = []
    for c in range(NCORES):
        b, gidx = c // 2, c % 2
        in_maps.append({
            "xq": xs_arr[("q", b)],
            "xk": xs_arr[("k", b)],
            "xv": xs_arr[("v", b)],
            "wq": ws[("wq", gidx)],
            "wk": ws[("wk", gidx)],
            "wv": ws[("wv", gidx)],
            "wo": ws[("wo", gidx)],
            "bq": ws[("bq", gidx)],
            "bk": ws[("bk", gidx)],
            "bv": ws[("bv", gidx)],
        })
    return in_maps


def run_hw(inputs, trace=False, trace_cores=None):
    nc = get_nc()
    in_maps = prepare_in_maps(**inputs)
    res = bass_utils.run_bass_kernel_spmd(
        nc, in_maps, core_ids=list(range(NCORES)),
        trace=trace, trace_cores=trace_cores,
    )
    bo = np.asarray(inputs["bo"], np.float32)
    out = np.empty((B, S, D), np.float32)
    for b in range(B):
        acc = res.results[2 * b]["out"] + res.results[2 * b + 1]["out"]
        out[b] = acc.T + bo
    return out, res


def kernel(**inputs):
    out, _ = run_hw(inputs, trace=False)
    return out


if __name__ == "__main__":
    rng = np.random.default_rng(0)
    ins = {
        "query": rng.standard_normal((B, S, D), np.float32),
        "key": rng.standard_normal((B, S, D), np.float32),
        "value": rng.standard_normal((B, S, D), np.float32),
        "Wq": (rng.standard_normal((D, D)) * D ** -0.5).astype(np.float32),
        "bq": np.zeros(D, np.float32),
        "Wk": (rng.standard_normal((D, D)) * D ** -0.5).astype(np.float32),
        "bk": np.zeros(D, np.float32),
        "Wv": (rng.standard_normal((D, D)) * D ** -0.5).astype(np.float32),
        "bv": np.zeros(D, np.float32),
        "Wo": (rng.standard_normal((D, D)) * D ** -0.5).astype(np.float32),
        "bo": np.zeros(D, np.float32),
    }
    out = kernel(**ins)
    print("kernel out", out.shape, out.dtype, float(np.abs(out).mean()))
